# revision 3
# baseline (speedup 1.0000x reference)
"""Causal attention head (B=8, S=4096, dk=64, scale=1/dk) on 8 TRN2 NeuronCores.

Data-parallel: batch b -> core b. Per core, flash-style causal attention
computed in [kv, q] orientation so that:
  - QK^T matmuls contract dk (on partitions) with q streaming (N=512)
  - softmax denominator comes free via a ones-column appended to v
  - PV matmuls contract kv (on partitions) with q streaming (N=512)
Scores never touch HBM; exp runs on ScalarE over 3-bank PSUM triples.
"""

import numpy as np
import ml_dtypes

B, S, DK = 8, 4096, 64
QB = 512           # q superblock width (PSUM bank = 512 fp32)
KB = 128           # kv chunk (partition dim)
NK = S // KB       # 32 kv chunks
NQ = S // QB       # 8 q superblocks
TR = 3             # kv chunks per exp "triple" (3 PSUM banks)

_cache = {}


def _build():
    from concourse.bacc import Bacc
    from concourse import tile, masks
    import concourse.mybir as mybir

    f32 = mybir.dt.float32
    bf16 = mybir.dt.bfloat16

    nc = Bacc(None, target_bir_lowering=False)
    qt_d = nc.dram_tensor("qt", [DK, S], bf16, kind="ExternalInput")
    kt_d = nc.dram_tensor("kt", [DK, S], bf16, kind="ExternalInput")
    vp_d = nc.dram_tensor("vp", [KB, NK, DK + 1], bf16, kind="ExternalInput")
    out_d = nc.dram_tensor("out", [S, DK], f32, kind="ExternalOutput")

    with tile.TileContext(nc) as tc:
        with (
            tc.tile_pool(name="const", bufs=1) as constp,
            tc.tile_pool(name="inp", bufs=1) as inp,
            tc.tile_pool(name="attn", bufs=3) as attnp,
            tc.tile_pool(name="nv", bufs=2) as nvp,
            tc.tile_pool(name="outp", bufs=2) as outp,
            tc.tile_pool(name="rp", bufs=2) as rpp,
            tc.tile_pool(name="qk_ps", bufs=2, space="PSUM") as qkps,
            tc.tile_pool(name="pv_ps", bufs=1, space="PSUM") as pvps,
            tc.tile_pool(name="tr_ps", bufs=1, space="PSUM") as trps,
        ):
            ident = constp.tile([128, 128], f32)
            masks.make_identity(nc, ident[:])

            # 4 causal masks: diag position d -> keep where qf - p - 128*d >= 0
            cmask = constp.tile([128, 4, QB], bf16)
            nc.gpsimd.memset(cmask[:], 1.0)
            for d in range(4):
                nc.gpsimd.affine_select(
                    out=cmask[:, d, :],
                    in_=cmask[:, d, :],
                    pattern=[[1, QB]],
                    compare_op=mybir.AluOpType.is_ge,
                    fill=0.0,
                    base=-KB * d,
                    channel_multiplier=-1,
                )

            qt = inp.tile([DK, S], bf16)
            kt = inp.tile([DK, S], bf16)
            vp = inp.tile([KB, NK, DK + 1], bf16)
            nc.sync.dma_start(out=qt[:], in_=qt_d[:])
            nc.sync.dma_start(out=kt[:], in_=kt_d[:])
            nc.sync.dma_start(out=vp[:], in_=vp_d[:])

            for I in range(NQ):
                C = 4 * I + 4          # causal kv chunks for this superblock
                pv = pvps.tile([DK + 1, QB], f32, tag="pv")
                q_rhs = qt[:, I * QB:(I + 1) * QB]
                j = 0
                while j < C:
                    n = min(TR, C - j)
                    qk = qkps.tile([128, TR, QB], f32, tag="qk")
                    at = attnp.tile([128, TR, QB], bf16, tag="at")
                    for u in range(n):
                        nc.tensor.matmul(
                            qk[:, u, :],
                            kt[:, (j + u) * KB:(j + u + 1) * KB],
                            q_rhs,
                            start=True, stop=True,
                        )
                    nc.scalar.activation(
                        out=at[:, 0:n, :],
                        in_=qk[:, 0:n, :],
                        func=mybir.ActivationFunctionType.Exp,
                        scale=1.0 / DK,
                    )
                    for u in range(n):
                        jj = j + u
                        if jj >= 4 * I:  # diagonal chunk: causal mask
                            nc.vector.tensor_mul(
                                at[:, u, :],
                                at[:, u, :],
                                cmask[:, jj - 4 * I, :],
                            )
                        nc.tensor.matmul(
                            pv[:],
                            vp[:, jj, :],
                            at[:, u, :],
                            start=(jj == 0), stop=(jj == C - 1),
                        )
                    j += n

                # epilogue: normalize + transpose back to [q, d]
                nv = nvp.tile([DK + 1, QB], f32, tag="nv")
                nc.vector.tensor_copy(nv[:], pv[:])
                ot = outp.tile([128, 4, DK], f32, tag="ot")
                for t in range(4):
                    trp = trps.tile([128, DK + 1], f32, tag="tr")
                    nc.tensor.matmul(
                        trp[:],
                        nv[:, t * 128:(t + 1) * 128],
                        ident[0:DK + 1, 0:DK + 1],
                        is_transpose=True,
                    )
                    r = rpp.tile([128, 1], f32, tag="r")
                    nc.vector.reciprocal(r[:], trp[:, DK:DK + 1])
                    nc.vector.tensor_scalar(
                        ot[:, t, :],
                        trp[:, 0:DK],
                        r[:],
                        None,
                        mybir.AluOpType.mult,
                    )
                nc.sync.dma_start(
                    out=out_d[I * QB:(I + 1) * QB].rearrange(
                        "(t p) d -> p t d", p=128
                    ),
                    in_=ot[:],
                )

    nc.compile()
    return nc


def _get_nc():
    if "nc" not in _cache:
        _cache["nc"] = _build()
    return _cache["nc"]


def kernel(q, k, v):
    from concourse.bass_utils import run_bass_kernel_spmd

    nc = _get_nc()
    bf = ml_dtypes.bfloat16
    q = np.asarray(q)
    k = np.asarray(k)
    v = np.asarray(v)

    in_maps = []
    for b in range(B):
        qt = np.ascontiguousarray(q[b].T).astype(bf)          # [64, 4096]
        kt = np.ascontiguousarray(k[b].T).astype(bf)          # [64, 4096]
        vpk = np.empty((KB, NK, DK + 1), dtype=bf)
        vpk[:, :, 0:DK] = v[b].reshape(NK, KB, DK).transpose(1, 0, 2)
        vpk[:, :, DK] = 1.0
        in_maps.append({"qt": qt, "kt": kt, "vp": vpk})

    res = run_bass_kernel_spmd(nc, in_maps, core_ids=list(range(B)))
    out = np.stack([np.asarray(res.results[i]["out"]) for i in range(B)], axis=0)
    return out.astype(np.float32)


# revision 4
# speedup vs baseline: 1.1576x; 1.1576x over previous
"""Causal attention head (B=8, S=4096, dk=64, scale=1/dk) on 8 TRN2 NeuronCores.

Data-parallel: batch b -> core b. Per core, flash-style causal attention
computed in [kv, q] orientation so that:
  - QK^T matmuls contract dk (on partitions) with q streaming (N=512),
    row-packed 2x via 64x128 PE tiling (T0 = SBUF partitions 0-63,
    T8 = partitions 64-127) since the contraction dim is only 64
  - softmax denominator comes free via a ones-column appended to v
  - PV matmuls contract kv (on partitions) with q streaming (N=512)
Scores never touch HBM; exp runs on ScalarE over 3-bank PSUM triples.

Host-side shard packing per batch:
  qtp [128, 4096] bf16 : q^T duplicated into both partition halves
  ktp [128, 16, 128] bf16 : k^T chunk 2m in partitions 0-63, 2m+1 in 64-127
  vp  [128, 32, 65] bf16 : v chunks (kv on partitions) + ones column
"""

import numpy as np
import ml_dtypes

B, S, DK = 8, 4096, 64
QB = 512           # q superblock width (PSUM bank = 512 fp32)
KB = 128           # kv chunk (partition dim)
NK = S // KB       # 32 kv chunks
NQ = S // QB       # 8 q superblocks
TR = 3             # kv chunks per exp "triple" (3 PSUM banks)

_cache = {}


def _build():
    from concourse.bacc import Bacc
    from concourse import tile, masks
    import concourse.mybir as mybir

    f32 = mybir.dt.float32
    bf16 = mybir.dt.bfloat16

    nc = Bacc(None, target_bir_lowering=False)
    qt_d = nc.dram_tensor("qtp", [128, S], bf16, kind="ExternalInput")
    kt_d = nc.dram_tensor("ktp", [128, NK // 2, KB], bf16, kind="ExternalInput")
    vp_d = nc.dram_tensor("vp", [KB, NK, DK + 1], bf16, kind="ExternalInput")
    out_d = nc.dram_tensor("out", [S, DK], f32, kind="ExternalOutput")

    with tile.TileContext(nc) as tc:
        with (
            tc.tile_pool(name="const", bufs=1) as constp,
            tc.tile_pool(name="inp", bufs=1) as inp,
            tc.tile_pool(name="attn", bufs=4) as attnp,
            tc.tile_pool(name="nv", bufs=2) as nvp,
            tc.tile_pool(name="outp", bufs=2) as outp,
            tc.tile_pool(name="rp", bufs=4) as rpp,
            tc.tile_pool(name="qk_ps", bufs=2, space="PSUM") as qkps,
            tc.tile_pool(name="pv_ps", bufs=1, space="PSUM") as pvps,
            tc.tile_pool(name="tr_ps", bufs=1, space="PSUM") as trps,
        ):
            ident = constp.tile([128, 128], f32)
            masks.make_identity(nc, ident[:])

            # 4 causal masks: diag position d -> keep where qf - p - 128*d >= 0
            cmask = constp.tile([128, 4, QB], bf16)
            nc.gpsimd.memset(cmask[:], 1.0)
            for d in range(4):
                nc.gpsimd.affine_select(
                    out=cmask[:, d, :],
                    in_=cmask[:, d, :],
                    pattern=[[1, QB]],
                    compare_op=mybir.AluOpType.is_ge,
                    fill=0.0,
                    base=-KB * d,
                    channel_multiplier=-1,
                )

            qt = inp.tile([128, S], bf16)
            kt = inp.tile([128, NK // 2, KB], bf16)
            vp = inp.tile([KB, NK, DK + 1], bf16)
            # prioritized chunked input DMAs: first superblock's operands
            # land early so PE starts ~4us in, remainder streams behind.
            nc.sync.dma_start(out=qt[:, 0:QB], in_=qt_d[:, 0:QB])
            nc.sync.dma_start(out=kt[:, 0:2], in_=kt_d[:, 0:2])
            nc.sync.dma_start(out=vp[:, 0:4], in_=vp_d[:, 0:4])
            nc.sync.dma_start(out=qt[:, QB:S], in_=qt_d[:, QB:S])
            nc.sync.dma_start(out=kt[:, 2:8], in_=kt_d[:, 2:8])
            nc.sync.dma_start(out=vp[:, 4:16], in_=vp_d[:, 4:16])
            nc.sync.dma_start(out=kt[:, 8:16], in_=kt_d[:, 8:16])
            nc.sync.dma_start(out=vp[:, 16:32], in_=vp_d[:, 16:32])

            for I in range(NQ):
                C = 4 * I + 4          # causal kv chunks for this superblock
                pv = pvps.tile([DK + 1, QB], f32, tag="pv")
                j = 0
                while j < C:
                    n = min(TR, C - j)
                    qk = qkps.tile([128, TR, QB], f32, tag="qk")
                    at = attnp.tile([128, TR, QB], bf16, tag="at")
                    for u in range(n):
                        jj = j + u
                        m, h = jj // 2, jj % 2
                        nc.tensor.matmul(
                            qk[:, u, :],
                            kt[h * 64:(h + 1) * 64, m, :],
                            qt[h * 64:(h + 1) * 64, I * QB:(I + 1) * QB],
                            start=True, stop=True,
                            tile_position=(64 * h, 0),
                        )
                    nc.scalar.activation(
                        out=at[:, 0:n, :],
                        in_=qk[:, 0:n, :],
                        func=mybir.ActivationFunctionType.Exp,
                        scale=1.0 / DK,
                    )
                    for u in range(n):
                        jj = j + u
                        if jj >= 4 * I:  # diagonal chunk: causal mask
                            nc.vector.tensor_mul(
                                at[:, u, :],
                                at[:, u, :],
                                cmask[:, jj - 4 * I, :],
                            )
                        nc.tensor.matmul(
                            pv[:],
                            vp[:, jj, :],
                            at[:, u, :],
                            start=(jj == 0), stop=(jj == C - 1),
                        )
                    j += n

                # epilogue: normalize + transpose back to [q, d]
                nv = nvp.tile([DK + 1, QB], f32, tag="nv")
                nc.vector.tensor_copy(nv[:], pv[:])
                ot = outp.tile([128, 4, DK], f32, tag="ot")
                for t in range(4):
                    trp = trps.tile([128, DK + 1], f32, tag="tr")
                    nc.tensor.matmul(
                        trp[:],
                        nv[:, t * 128:(t + 1) * 128],
                        ident[0:DK + 1, 0:DK + 1],
                        is_transpose=True,
                    )
                    r = rpp.tile([128, 1], f32, tag="r")
                    nc.vector.reciprocal(r[:], trp[:, DK:DK + 1])
                    nc.vector.tensor_scalar(
                        ot[:, t, :],
                        trp[:, 0:DK],
                        r[:],
                        None,
                        mybir.AluOpType.mult,
                    )
                nc.sync.dma_start(
                    out=out_d[I * QB:(I + 1) * QB].rearrange(
                        "(t p) d -> p t d", p=128
                    ),
                    in_=ot[:],
                )

    nc.compile()
    return nc


def _get_nc():
    if "nc" not in _cache:
        _cache["nc"] = _build()
    return _cache["nc"]


def make_in_maps(q, k, v):
    bf = ml_dtypes.bfloat16
    q = np.asarray(q)
    k = np.asarray(k)
    v = np.asarray(v)
    in_maps = []
    for b in range(B):
        qt = np.ascontiguousarray(q[b].T).astype(bf)          # [64, 4096]
        qtp = np.concatenate([qt, qt], axis=0)                # [128, 4096]
        kt = np.ascontiguousarray(k[b].T).astype(bf)          # [64, 4096]
        ktc = kt.reshape(DK, NK, KB)                          # [64, 32, 128]
        ktp = np.empty((128, NK // 2, KB), dtype=bf)
        ktp[0:DK] = ktc[:, 0::2, :]
        ktp[DK:128] = ktc[:, 1::2, :]
        vpk = np.empty((KB, NK, DK + 1), dtype=bf)
        vpk[:, :, 0:DK] = v[b].reshape(NK, KB, DK).transpose(1, 0, 2)
        vpk[:, :, DK] = 1.0
        in_maps.append({"qtp": qtp, "ktp": np.ascontiguousarray(ktp),
                        "vp": vpk})
    return in_maps


def kernel(q, k, v):
    from concourse.bass_utils import run_bass_kernel_spmd

    nc = _get_nc()
    in_maps = make_in_maps(q, k, v)
    res = run_bass_kernel_spmd(nc, in_maps, core_ids=list(range(B)))
    out = np.stack([np.asarray(res.results[i]["out"]) for i in range(B)], axis=0)
    return out.astype(np.float32)
